# revision 1
# baseline (speedup 1.0000x reference)
"""CompressedIndicatorEmbedding kernel for 8 Trainium2 NeuronCores.

out[n] = sum_p W[:, p*512 + x[n, p]]  for x [N=1048576, 8] int32, W [64, 4096] f32.

Strategy: data-parallel over tokens (N/8 per core). Per core, the lookup is
computed as 32 PSUM-accumulated matmuls per 128-token tile:
  psum[128 tok, 64] += OH_j[128 codes, 128 tok].T @ Wchunk_j[128 codes, 64]
where OH_j is the exact indicator (one-hot) built on the Vector engine with a
tensor_scalar is_equal against a per-partition iota, in fp16 (codes < 2048 are
exact in fp16, so the indicator is exact; W is fp16, rel err ~3e-4).
"""
import sys
sys.path.insert(0, "/opt/trn_rl_repo")
import numpy as np
import concourse.bacc as bacc
import concourse.bass as bass
import concourse.mybir as mybir
from concourse.tile import TileContext
from concourse.bass_utils import run_bass_kernel_spmd

N_CORES = 8
N = 1048576
P = 8
L = 512
D = 64
T = N // N_CORES          # tokens per core
BLK = 1024                # tokens per loop iteration
NB = T // BLK
SUB = BLK // 128
F32, F16 = mybir.dt.float32, mybir.dt.float16
PE = mybir.EngineType.PE

_CACHED_NC = None


def _build():
    nc = bacc.Bacc("TRN2", target_bir_lowering=False, debug=False,
                   enable_asserts=False, num_devices=1)
    xt = nc.dram_tensor("xt", [P, T], F16, kind="ExternalInput")
    wt = nc.dram_tensor("wt", [128, 2048], F16, kind="ExternalInput")
    iot = nc.dram_tensor("iot", [128, 4], F32, kind="ExternalInput")
    out = nc.dram_tensor("out", [T, D], F32, kind="ExternalOutput")

    with TileContext(nc) as tc:
        with tc.tile_pool(name="const", bufs=1) as cpool, \
             tc.tile_pool(name="xrep", bufs=3) as rpool, \
             tc.tile_pool(name="oh", bufs=6) as opool, \
             tc.tile_pool(name="psum", bufs=1, space="PSUM") as ppool, \
             tc.tile_pool(name="osb", bufs=3) as spool:
            w = cpool.tile([128, 2048], F16)
            nc.sync.dma_start(w[:], wt[:])
            io = cpool.tile([128, 4], F32)
            nc.sync.dma_start(io[:], iot[:])

            def body(i):
                xreps = []
                for p in range(P):
                    xr = rpool.tile([128, BLK], F16, tag=f"xrep{p % 3}",
                                    name=f"xr{p}")
                    nc.sync.dma_start(
                        xr[:],
                        xt[p:p + 1, bass.ts(i, BLK)].to_broadcast([128, BLK]))
                    xreps.append(xr)
                psums = [ppool.tile([128, 64], F32, tag=f"ps{s}", name=f"ps{s}")
                         for s in range(SUB)]
                for j in range(32):
                    p, k = divmod(j, 4)
                    oh = opool.tile([128, BLK], F16, tag="oh", name="oh")
                    nc.vector.tensor_scalar(
                        oh[:], xreps[p][:], io[:, k:k + 1], None,
                        mybir.AluOpType.is_equal)
                    for s in range(SUB):
                        nc.tensor.matmul(
                            psums[s][:],
                            oh[:, s * 128:(s + 1) * 128],
                            w[:, j * 64:(j + 1) * 64],
                            start=(j == 0), stop=(j == 31))
                for s in range(SUB):
                    ot = spool.tile([128, 64], F32, tag="ot", name="ot")
                    nc.scalar.copy(ot[:], psums[s][:])
                    nc.sync.dma_start(
                        out[bass.ds(i * BLK + s * 128, 128), :], ot[:])

            with tc.For_i(0, NB, 1, hint_engines=(PE,),
                          staggered_reset=True) as i:
                body(i)
    nc.compile()
    return nc


def kernel(x: np.ndarray, W: np.ndarray) -> np.ndarray:
    global _CACHED_NC
    assert x.shape == (N, P) and W.shape == (D, P * L)
    if _CACHED_NC is None:
        _CACHED_NC = _build()
    nc = _CACHED_NC

    # host-side input prep (layout/sharding only)
    wt = np.ascontiguousarray(W.T).astype(np.float16)          # [4096, 64]
    wt = wt.reshape(32, 128, 64).transpose(1, 0, 2).reshape(128, 2048)
    wt = np.ascontiguousarray(wt)
    iot = (np.arange(128)[:, None] +
           128 * np.arange(4)[None, :]).astype(np.float32)
    in_maps = []
    for c in range(N_CORES):
        xc = x[c * T:(c + 1) * T]                              # [T, 8] int32
        xtc = np.ascontiguousarray(xc.T).astype(np.float16)    # [8, T]
        in_maps.append({"xt": xtc, "wt": wt, "iot": iot})

    res = run_bass_kernel_spmd(nc, in_maps, core_ids=list(range(N_CORES)))
    return np.concatenate(
        [res.results[c]["out"] for c in range(N_CORES)], axis=0)

